# revision 5
# baseline (speedup 1.0000x reference)
"""DeepClusteringLoss Trainium2 kernel.

loss = (||V^T V||_F^2 - 2 ||V^T E||_F^2 + ||E^T E||_F^2) / (B*N)
summed over batch, with E = embeddings.reshape(B, N, D), V =
assignments.reshape(B, N, S), N = F*T.

Sharding: data-parallel over batch; one core per batch element; the host
sums the 8 per-core partials (the scalar "all-reduce") and divides by
B*N.

Per-core pipeline (DMA/HBM-bound: 23.07 MB fp32 input @ ~358 GB/s/core
=> ~64.4 us transfer floor):
- GLOBAL partition map: partition p owns rows [p*1024, (p+1)*1024).
  Chunk c = column c of every partition = 128 rows.
- ALL streaming is HWDGE (SP + ACT rings) in fp32: HWDGE descriptor
  generation is RTL (no Q7 SWDGE boot delay, which cost ~3-6 us of
  16-engine idle at the front), every DMA sprays all 16 SDMA engines
  evenly (the SWDGE baseline left engines 11-15 ~6 us underloaded), and
  since HBM (~358 GB/s) binds before the SBUF AXI fabric (435 GB/s),
  streaming fp32 instead of cast-to-fp16 costs no bandwidth.
- V (2 MB) goes first on the ACT ring into a resident fp32 tile; E
  streams as 19 tapered column-slices (14x64 + 48,32,24,16,8 chunks)
  alternating SP/ACT rings through an 8-deep fp32 ring buffer.
- Interleave copies (DVE for E, ACT for V) cast fp32->fp16 while
  building chunk-PAIR operands [V_2q | E_2q | pad20 | V_2q+1 | E_2q+1]
  (128 x 108 fp16): ONE matmul per two chunks -> 512 PE instruction
  pairs.  Even/odd Grams accumulate at PSUM partition bases 0/64; pad
  and cross-term cells are never read.
- Epilogue dumps the two 44x44 diagonal Gram blocks (SP + ACT rings in
  parallel); the host adds them and reduces to the scalar partial in
  float64 (exact).
"""

import os
from contextlib import ExitStack

import numpy as np

import concourse.bacc as bacc
import concourse.mybir as mybir
import concourse.tile as tile
from concourse.bass_utils import run_bass_kernel_spmd

B, F, T, D, S = 8, 256, 512, 40, 4
N = F * T              # rows per core (131072)
SD = S + D             # 44 combined features
PW = 108               # paired-chunk width: 44 | 20 pad | 44
P = 128                # partitions
U = N // P             # rows per partition in the global map (1024)
N_CORES = 8

MM_DT_NAME = os.environ.get("KERNEL_MM_DT", "float16")
RING = os.environ.get("KERNEL_RING", "alt")   # "alt" | "sp"
EBUFS = int(os.environ.get("KERNEL_EBUFS", "10"))
WBUFS = int(os.environ.get("KERNEL_WBUFS", "6"))

# E slice taper: big uniform slices for line-rate DMA, small tail so the
# last-slice copy+matmul+epilogue dependency chain is short.
SLICES = [64] * 14 + [48, 32, 24, 16, 8]
assert sum(SLICES) == U
assert all(ub % 2 == 0 for ub in SLICES)

_nc_cache = {}


def _build_nc(key):
    (mm_dt_name, ring_mode, ebufs, wbufs) = key
    mm_dt = getattr(mybir.dt, mm_dt_name)
    f32 = mybir.dt.float32

    nc = bacc.Bacc("TRN2", target_bir_lowering=False, debug=False)
    E = nc.dram_tensor("embeddings", (N, D), f32, kind="ExternalInput")
    V = nc.dram_tensor("assignments", (N, S), f32, kind="ExternalInput")
    OUT = nc.dram_tensor("partial", (PW, PW), f32, kind="ExternalOutput")

    # global-map DRAM views: partition p <- rows [p*U, (p+1)*U)
    e_g = E[:, :].rearrange("(p u) d -> p (u d)", p=P)   # [128, U*D]
    v_g = V[:, :].rearrange("(p u) s -> p (u s)", p=P)   # [128, U*S]

    with tile.TileContext(nc) as tc, ExitStack() as ctx:
        res_pool = ctx.enter_context(tc.tile_pool(name="res", bufs=1))
        e_pool = ctx.enter_context(tc.tile_pool(name="e", bufs=ebufs))
        w_pool = ctx.enter_context(tc.tile_pool(name="w", bufs=wbufs))
        psum_pool = ctx.enter_context(tc.tile_pool(name="ps", bufs=1, space="PSUM"))
        g_ps = psum_pool.tile([PW, PW], f32, tag="g")

        # V up front: one 2 MB fp32 HWDGE DMA on the ACT ring into a
        # resident tile; the ACT interleave copies cast it later.
        v_all = res_pool.tile([P, U * S], f32, tag="v")
        nc.scalar.dma_start(out=v_all[:], in_=v_g)

        pair = 0
        n_pairs = U // 2
        c0 = 0
        for k, ub in enumerate(SLICES):
            last = k == len(SLICES) - 1
            # E slice: fp32 HWDGE DMA into one of `ebufs` ring slots.
            e_t = e_pool.tile([P, ub * D], f32, tag="e")
            eng = nc.sync if (ring_mode == "sp" or k % 2 == 0) else nc.scalar
            eng.dma_start(out=e_t[:], in_=e_g[:, c0 * D:(c0 + ub) * D])

            nq = ub // 2
            w_t = w_pool.tile([P, nq * PW], mm_dt, tag="w")
            w4 = w_t[:].rearrange("p (q c) -> p q c", c=PW)
            e2 = e_t[:].rearrange("p (q r) -> p q r", r=2 * D)
            v2 = v_all[:, c0 * S:(c0 + ub) * S].rearrange(
                "p (q r) -> p q r", r=2 * S)
            # All four interleave copies run on DVE: the ACT sequencer
            # must stay DMA-only, or its in-order stream blocks tail E
            # DMA issue behind PE-gated copy waits.
            nc.vector.tensor_copy(w4[:, :, S:SD], e2[:, :, 0:D])
            nc.vector.tensor_copy(w4[:, :, 64 + S:64 + SD], e2[:, :, D:2 * D])
            nc.vector.tensor_copy(w4[:, :, 0:S], v2[:, :, 0:S])
            nc.vector.tensor_copy(w4[:, :, 64:64 + S], v2[:, :, S:2 * S])
            for q in range(nq):
                wq = w_t[:, q * PW:(q + 1) * PW]
                nc.tensor.matmul(
                    g_ps[:], wq, wq,
                    start=(pair == 0),
                    stop=(last and q == nq - 1),
                )
                pair += 1
            c0 += ub

        # Epilogue: dump only the two 44x44 diagonal Gram blocks of the
        # PSUM accumulator, each on its own HWDGE ring (SP and ACT) so
        # the descriptor generation for the two OUT transfers runs in
        # parallel; the host adds the blocks and reduces to the scalar
        # partial (exact, in float64) alongside the cross-core sum.
        ep = ctx.enter_context(tc.tile_pool(name="ep", bufs=1))
        ge_sb = ep.tile([SD, SD], f32, tag="ge")
        go_sb = ep.tile([64 + SD, SD], f32, tag="go")
        nc.vector.tensor_copy(ge_sb[:], g_ps[0:SD, 0:SD])
        nc.scalar.copy(
            go_sb[64:64 + SD, :], g_ps[64:64 + SD, 64:64 + SD])
        nc.sync.dma_start(out=OUT[0:SD, 0:SD], in_=ge_sb[:])
        nc.scalar.dma_start(
            out=OUT[64:64 + SD, 64:64 + SD], in_=go_sb[64:64 + SD, :])

    nc.finalize()
    return nc


def _get_nc():
    key = (MM_DT_NAME, RING, EBUFS, WBUFS)
    if key not in _nc_cache:
        _nc_cache[key] = _build_nc(key)
    return _nc_cache[key]


def _run(embeddings: np.ndarray, assignments: np.ndarray, trace: bool = False):
    nc = _get_nc()
    in_maps = []
    for i in range(N_CORES):
        in_maps.append({
            "embeddings": np.ascontiguousarray(
                embeddings[i].reshape(N, D).astype(np.float32, copy=False)),
            "assignments": np.ascontiguousarray(
                assignments[i].reshape(N, S).astype(np.float32, copy=False)),
        })
    try:
        res = run_bass_kernel_spmd(
            nc, in_maps, core_ids=list(range(N_CORES)), trace=trace
        )
    except Exception:
        res = run_bass_kernel_spmd(
            nc, in_maps, core_ids=list(range(N_CORES)), trace=trace
        )
    partials = []
    for r in res.results:
        gp = np.asarray(r["partial"], dtype=np.float64)
        G = gp[0:SD, 0:SD] + gp[64:64 + SD, 64:64 + SD]
        bm = G[0:S, S:SD]
        partials.append(np.sum(G * G) - 4.0 * np.sum(bm * bm))
    total = np.float32(np.sum(np.asarray(partials, dtype=np.float64)) / (B * N))
    return np.asarray(total, dtype=np.float32), res


def kernel(embeddings: np.ndarray, assignments: np.ndarray) -> np.ndarray:
    out, _ = _run(embeddings, assignments, trace=False)
    return out


# revision 11
# speedup vs baseline: 1.0731x; 1.0731x over previous
"""DeepClusteringLoss Trainium2 kernel.

loss = (||V^T V||_F^2 - 2 ||V^T E||_F^2 + ||E^T E||_F^2) / (B*N)
summed over batch, with E = embeddings.reshape(B, N, D), V =
assignments.reshape(B, N, S), N = F*T.

Sharding: data-parallel over batch; one core per batch element; the host
sums the 8 per-core partials (the scalar "all-reduce") and divides by
B*N.

Per-core pipeline (DMA/HBM-bound: 23.07 MB fp32 input @ ~358 GB/s/core
=> ~64.4 us transfer floor):
- GLOBAL partition map: partition p owns rows [p*1024, (p+1)*1024).
  Chunk c = column c of every partition = 128 rows.
- ALL streaming is HWDGE (SP + ACT rings) in fp32: HWDGE descriptor
  generation is RTL (no Q7 SWDGE boot delay, which cost ~3-6 us of
  16-engine idle at the front), every DMA sprays all 16 SDMA engines
  evenly (the SWDGE baseline left engines 11-15 ~6 us underloaded), and
  since HBM (~358 GB/s) binds before the SBUF AXI fabric (435 GB/s),
  streaming fp32 instead of cast-to-fp16 costs no bandwidth.
- V (2 MB) goes first on the ACT ring into a resident fp32 tile; E
  streams as 19 tapered column-slices (14x64 + 48,32,24,16,8 chunks)
  alternating SP/ACT rings through an 8-deep fp32 ring buffer.
- Interleave copies (DVE for E, ACT for V) cast fp32->fp16 while
  building chunk-PAIR operands [V_2q | E_2q | pad20 | V_2q+1 | E_2q+1]
  (128 x 108 fp16): ONE matmul per two chunks -> 512 PE instruction
  pairs.  Even/odd Grams accumulate at PSUM partition bases 0/64; pad
  and cross-term cells are never read.
- Epilogue dumps the two 44x44 diagonal Gram blocks (SP + ACT rings in
  parallel); the host adds them and reduces to the scalar partial in
  float64 (exact).
"""

import os
from contextlib import ExitStack

import numpy as np

import concourse.bacc as bacc
import concourse.mybir as mybir
import concourse.tile as tile
from concourse.bass_utils import run_bass_kernel_spmd

B, F, T, D, S = 8, 256, 512, 40, 4
N = F * T              # rows per core (131072)
SD = S + D             # 44 combined features
H = 48                 # half-width: V(4) | E(40) | pad(4); 48*2B = 32B-aligned
PW = 2 * H             # paired-chunk width (96)
P = 128                # partitions
U = N // P             # rows per partition in the global map (1024)
N_CORES = 8

MM_DT_NAME = os.environ.get("KERNEL_MM_DT", "float16")
RING = os.environ.get("KERNEL_RING", "alt")   # "alt" | "sp"
EBUFS = int(os.environ.get("KERNEL_EBUFS", "10"))
WBUFS = int(os.environ.get("KERNEL_WBUFS", "6"))

# E slice taper: big uniform slices for line-rate DMA, small tail so the
# last-slice copy+matmul+epilogue dependency chain is short.
SLICES = [64] * 14 + [48, 32, 24, 16, 8]
assert sum(SLICES) == U
assert all(ub % 2 == 0 for ub in SLICES)

_nc_cache = {}


def _build_nc(key):
    (mm_dt_name, ring_mode, ebufs, wbufs) = key
    mm_dt = getattr(mybir.dt, mm_dt_name)
    f32 = mybir.dt.float32

    nc = bacc.Bacc("TRN2", target_bir_lowering=False, debug=False)
    E = nc.dram_tensor("embeddings", (N, D), f32, kind="ExternalInput")
    V = nc.dram_tensor("assignments", (N, S), f32, kind="ExternalInput")
    OUT = nc.dram_tensor("partial", (PW, PW), f32, kind="ExternalOutput")

    # global-map DRAM views: partition p <- rows [p*U, (p+1)*U)
    e_g = E[:, :].rearrange("(p u) d -> p (u d)", p=P)   # [128, U*D]
    v_g = V[:, :].rearrange("(p u) s -> p (u s)", p=P)   # [128, U*S]

    with tile.TileContext(nc) as tc, ExitStack() as ctx:
        res_pool = ctx.enter_context(tc.tile_pool(name="res", bufs=1))
        e_pool = ctx.enter_context(tc.tile_pool(name="e", bufs=ebufs))
        w_pool = ctx.enter_context(tc.tile_pool(name="w", bufs=wbufs))
        psum_pool = ctx.enter_context(tc.tile_pool(name="ps", bufs=1, space="PSUM"))
        g_ps = psum_pool.tile([PW, PW], f32, tag="g")

        # V up front: one 2 MB fp32 HWDGE DMA on the ACT ring into a
        # resident tile; the ACT interleave copies cast it later.
        v_all = res_pool.tile([P, U * S], f32, tag="v")
        nc.scalar.dma_start(out=v_all[:], in_=v_g)

        pair = 0
        n_pairs = U // 2
        c0 = 0
        for k, ub in enumerate(SLICES):
            last = k == len(SLICES) - 1
            # E slice: fp32 HWDGE DMA into one of `ebufs` ring slots.
            e_t = e_pool.tile([P, ub * D], f32, tag="e")
            eng = nc.sync if (ring_mode == "sp" or k % 2 == 0) else nc.scalar
            eng.dma_start(out=e_t[:], in_=e_g[:, c0 * D:(c0 + ub) * D])

            nq = ub // 2
            w_t = w_pool.tile([P, nq * PW], mm_dt, tag="w")
            # 4D views: one cast per slice fills BOTH halves of every
            # pair.  Copies run on DVE only: the ACT sequencer must stay
            # DMA-only, or its in-order stream blocks tail E DMA issue
            # behind PE-gated copy waits.
            w5 = w_t[:].rearrange("p (q h c) -> p q h c", h=2, c=H)
            e3 = e_t[:].rearrange("p (q h d) -> p q h d", h=2, d=D)
            v3 = v_all[:, c0 * S:(c0 + ub) * S].rearrange(
                "p (q h s) -> p q h s", h=2, s=S)
            nc.vector.tensor_copy(w5[:, :, :, S:SD], e3)
            nc.vector.tensor_copy(w5[:, :, :, 0:S], v3)
            for q in range(nq):
                wq = w_t[:, q * PW:(q + 1) * PW]
                nc.tensor.matmul(
                    g_ps[:], wq, wq,
                    start=(pair == 0),
                    stop=(last and q == nq - 1),
                )
                pair += 1
            c0 += ub

        # Epilogue: dump only the two 44x44 diagonal Gram blocks of the
        # PSUM accumulator, each on its own HWDGE ring (SP and ACT) so
        # the descriptor generation for the two OUT transfers runs in
        # parallel; the host adds the blocks and reduces to the scalar
        # partial (exact, in float64) alongside the cross-core sum.
        # Partition-start legality: patterns may start at 0/32/64 and,
        # when starting at 32, cover at most 32 partitions.  The odd
        # Gram block lives at [48:92, 48:92], so dump it as two pieces:
        # rows 48:64 ride a 32-partition access at base 32, rows 64:92
        # a 28-partition access at base 64.
        ep = ctx.enter_context(tc.tile_pool(name="ep", bufs=1))
        ge_sb = ep.tile([SD, SD], f32, tag="ge")
        gl_sb = ep.tile([64, SD], f32, tag="gl")
        gh_sb = ep.tile([92, SD], f32, tag="gh")
        nc.vector.tensor_copy(ge_sb[:], g_ps[0:SD, 0:SD])
        nc.scalar.copy(gl_sb[32:64, :], g_ps[32:64, H:H + SD])
        nc.vector.tensor_copy(gh_sb[64:92, :], g_ps[64:92, H:H + SD])
        nc.sync.dma_start(out=OUT[0:SD, 0:SD], in_=ge_sb[:])
        nc.scalar.dma_start(out=OUT[32:64, H:H + SD], in_=gl_sb[32:64, :])
        nc.sync.dma_start(out=OUT[64:92, H:H + SD], in_=gh_sb[64:92, :])

    nc.finalize()
    return nc


def _get_nc():
    key = (MM_DT_NAME, RING, EBUFS, WBUFS)
    if key not in _nc_cache:
        _nc_cache[key] = _build_nc(key)
    return _nc_cache[key]


def _run(embeddings: np.ndarray, assignments: np.ndarray, trace: bool = False):
    nc = _get_nc()
    in_maps = []
    for i in range(N_CORES):
        in_maps.append({
            "embeddings": np.ascontiguousarray(
                embeddings[i].reshape(N, D).astype(np.float32, copy=False)),
            "assignments": np.ascontiguousarray(
                assignments[i].reshape(N, S).astype(np.float32, copy=False)),
        })
    try:
        res = run_bass_kernel_spmd(
            nc, in_maps, core_ids=list(range(N_CORES)), trace=trace
        )
    except Exception:
        res = run_bass_kernel_spmd(
            nc, in_maps, core_ids=list(range(N_CORES)), trace=trace
        )
    partials = []
    for r in res.results:
        gp = np.asarray(r["partial"], dtype=np.float64)
        G = gp[0:SD, 0:SD] + gp[H:H + SD, H:H + SD]
        bm = G[0:S, S:SD]
        partials.append(np.sum(G * G) - 4.0 * np.sum(bm * bm))
    total = np.float32(np.sum(np.asarray(partials, dtype=np.float64)) / (B * N))
    return np.asarray(total, dtype=np.float32), res


def kernel(embeddings: np.ndarray, assignments: np.ndarray) -> np.ndarray:
    out, _ = _run(embeddings, assignments, trace=False)
    return out


# revision 14
# speedup vs baseline: 1.0922x; 1.0179x over previous
"""DeepClusteringLoss Trainium2 kernel.

loss = (||V^T V||_F^2 - 2 ||V^T E||_F^2 + ||E^T E||_F^2) / (B*N)
summed over batch, with E = embeddings.reshape(B, N, D), V =
assignments.reshape(B, N, S), N = F*T.

Sharding: data-parallel over batch; one core per batch element; the host
sums the 8 per-core partials (the scalar "all-reduce") and divides by
B*N.

Per-core pipeline (DMA/HBM-bound: 23.07 MB fp32 input @ ~358 GB/s/core
=> ~64.4 us transfer floor):
- GLOBAL partition map: partition p owns rows [p*1024, (p+1)*1024).
  Chunk c = column c of every partition = 128 rows.
- ALL streaming is HWDGE (SP + ACT rings) in fp32: HWDGE descriptor
  generation is RTL (no Q7 SWDGE boot delay, which cost ~3-6 us of
  16-engine idle at the front), every DMA sprays all 16 SDMA engines
  evenly (the SWDGE baseline left engines 11-15 ~6 us underloaded), and
  since HBM (~358 GB/s) binds before the SBUF AXI fabric (435 GB/s),
  streaming fp32 instead of cast-to-fp16 costs no bandwidth.
- V (2 MB) goes first on the ACT ring into a resident fp32 tile; E
  streams as 19 tapered column-slices (14x64 + 48,32,24,16,8 chunks)
  alternating SP/ACT rings through an 8-deep fp32 ring buffer.
- Interleave copies (DVE for E, ACT for V) cast fp32->fp16 while
  building chunk-PAIR operands [V_2q | E_2q | pad20 | V_2q+1 | E_2q+1]
  (128 x 108 fp16): ONE matmul per two chunks -> 512 PE instruction
  pairs.  Even/odd Grams accumulate at PSUM partition bases 0/64; pad
  and cross-term cells are never read.
- Epilogue dumps the two 44x44 diagonal Gram blocks (SP + ACT rings in
  parallel); the host adds them and reduces to the scalar partial in
  float64 (exact).
"""

import os
from contextlib import ExitStack

import numpy as np

import concourse.bacc as bacc
import concourse.mybir as mybir
import concourse.tile as tile
from concourse.bass_utils import run_bass_kernel_spmd

B, F, T, D, S = 8, 256, 512, 40, 4
N = F * T              # rows per core (131072)
SD = S + D             # 44 combined features
H = 48                 # half-width: V(4) | E(40) | pad(4); 48*2B = 32B-aligned
PW = 2 * H             # paired-chunk width (96)
P = 128                # partitions
U = N // P             # rows per partition in the global map (1024)
N_CORES = 8

MM_DT_NAME = os.environ.get("KERNEL_MM_DT", "float16")
RING = os.environ.get("KERNEL_RING", "alt")   # "alt" | "sp"
EBUFS = int(os.environ.get("KERNEL_EBUFS", "10"))
WBUFS = int(os.environ.get("KERNEL_WBUFS", "6"))

# E slice taper: big uniform slices for line-rate DMA, small tail so the
# last-slice copy+matmul+epilogue dependency chain is short.
SLICES = [64] * 14 + [48, 32, 24, 16, 8]
assert sum(SLICES) == U
assert all(ub % 2 == 0 for ub in SLICES)

_nc_cache = {}


def _build_nc(key):
    (mm_dt_name, ring_mode, ebufs, wbufs) = key
    mm_dt = getattr(mybir.dt, mm_dt_name)
    f32 = mybir.dt.float32

    nc = bacc.Bacc("TRN2", target_bir_lowering=False, debug=False)
    E = nc.dram_tensor("embeddings", (N, D), f32, kind="ExternalInput")
    V = nc.dram_tensor("assignments", (N, S), f32, kind="ExternalInput")
    OUT = nc.dram_tensor("partial", (PW, PW), f32, kind="ExternalOutput")

    # global-map DRAM views: partition p <- rows [p*U, (p+1)*U)
    e_g = E[:, :].rearrange("(p u) d -> p (u d)", p=P)   # [128, U*D]
    v_g = V[:, :].rearrange("(p u) s -> p (u s)", p=P)   # [128, U*S]

    with tile.TileContext(nc) as tc, ExitStack() as ctx:
        res_pool = ctx.enter_context(tc.tile_pool(name="res", bufs=1))
        e_pool = ctx.enter_context(tc.tile_pool(name="e", bufs=ebufs))
        w_pool = ctx.enter_context(tc.tile_pool(name="w", bufs=wbufs))
        psum_pool = ctx.enter_context(tc.tile_pool(name="ps", bufs=1, space="PSUM"))
        g_ps = psum_pool.tile([PW, PW], f32, tag="g")

        # V up front in two 1 MB fp32 HWDGE pieces, one at the head of
        # each ring: piece 0 (chunks 0-511) lands by ~7us so the first
        # interleave copies and matmuls start early instead of waiting
        # ~18us for a single 2 MB transfer to drain.  Separate tiles so
        # early slices depend only on piece 0.
        VH = U // 2
        v_lo = res_pool.tile([P, VH * S], f32, tag="vlo")
        v_hi = res_pool.tile([P, VH * S], f32, tag="vhi")
        nc.sync.dma_start(out=v_lo[:], in_=v_g[:, :VH * S])
        nc.scalar.dma_start(out=v_hi[:], in_=v_g[:, VH * S:])

        pair = 0
        n_pairs = U // 2
        c0 = 0
        for k, ub in enumerate(SLICES):
            last = k == len(SLICES) - 1
            # E slice: fp32 HWDGE DMA into one of `ebufs` ring slots.
            e_t = e_pool.tile([P, ub * D], f32, tag="e")
            eng = nc.sync if (ring_mode == "sp" or k % 2 == 0) else nc.scalar
            eng.dma_start(out=e_t[:], in_=e_g[:, c0 * D:(c0 + ub) * D])

            nq = ub // 2
            w_t = w_pool.tile([P, nq * PW], mm_dt, tag="w")
            # 4D views: one cast per slice fills BOTH halves of every
            # pair.  Copies run on DVE only: the ACT sequencer must stay
            # DMA-only, or its in-order stream blocks tail E DMA issue
            # behind PE-gated copy waits.
            w5 = w_t[:].rearrange("p (q h c) -> p q h c", h=2, c=H)
            e3 = e_t[:].rearrange("p (q h d) -> p q h d", h=2, d=D)
            v_src = v_lo if c0 < VH else v_hi
            vc0 = c0 if c0 < VH else c0 - VH
            assert vc0 + ub <= VH
            v3 = v_src[:, vc0 * S:(vc0 + ub) * S].rearrange(
                "p (q h s) -> p q h s", h=2, s=S)
            nc.vector.tensor_copy(w5[:, :, :, S:SD], e3)
            nc.vector.tensor_copy(w5[:, :, :, 0:S], v3)
            for q in range(nq):
                wq = w_t[:, q * PW:(q + 1) * PW]
                nc.tensor.matmul(
                    g_ps[:], wq, wq,
                    start=(pair == 0),
                    stop=(last and q == nq - 1),
                )
                pair += 1
            c0 += ub

        # Epilogue: dump only the two 44x44 diagonal Gram blocks of the
        # PSUM accumulator, each on its own HWDGE ring (SP and ACT) so
        # the descriptor generation for the two OUT transfers runs in
        # parallel; the host adds the blocks and reduces to the scalar
        # partial (exact, in float64) alongside the cross-core sum.
        # Partition-start legality: patterns may start at 0/32/64 and,
        # when starting at 32, cover at most 32 partitions.  The odd
        # Gram block lives at [48:92, 48:92], so dump it as two pieces:
        # rows 48:64 ride a 32-partition access at base 32, rows 64:92
        # a 28-partition access at base 64.
        ep = ctx.enter_context(tc.tile_pool(name="ep", bufs=1))
        ge_sb = ep.tile([SD, SD], f32, tag="ge")
        gl_sb = ep.tile([64, SD], f32, tag="gl")
        gh_sb = ep.tile([92, SD], f32, tag="gh")
        nc.vector.tensor_copy(ge_sb[:], g_ps[0:SD, 0:SD])
        nc.scalar.copy(gl_sb[32:64, :], g_ps[32:64, H:H + SD])
        nc.vector.tensor_copy(gh_sb[64:92, :], g_ps[64:92, H:H + SD])
        nc.sync.dma_start(out=OUT[0:SD, 0:SD], in_=ge_sb[:])
        nc.scalar.dma_start(out=OUT[32:64, H:H + SD], in_=gl_sb[32:64, :])
        nc.sync.dma_start(out=OUT[64:92, H:H + SD], in_=gh_sb[64:92, :])

    nc.finalize()
    return nc


def _get_nc():
    key = (MM_DT_NAME, RING, EBUFS, WBUFS)
    if key not in _nc_cache:
        _nc_cache[key] = _build_nc(key)
    return _nc_cache[key]


def _run(embeddings: np.ndarray, assignments: np.ndarray, trace: bool = False):
    nc = _get_nc()
    in_maps = []
    for i in range(N_CORES):
        in_maps.append({
            "embeddings": np.ascontiguousarray(
                embeddings[i].reshape(N, D).astype(np.float32, copy=False)),
            "assignments": np.ascontiguousarray(
                assignments[i].reshape(N, S).astype(np.float32, copy=False)),
        })
    try:
        res = run_bass_kernel_spmd(
            nc, in_maps, core_ids=list(range(N_CORES)), trace=trace
        )
    except Exception:
        res = run_bass_kernel_spmd(
            nc, in_maps, core_ids=list(range(N_CORES)), trace=trace
        )
    partials = []
    for r in res.results:
        gp = np.asarray(r["partial"], dtype=np.float64)
        G = gp[0:SD, 0:SD] + gp[H:H + SD, H:H + SD]
        bm = G[0:S, S:SD]
        partials.append(np.sum(G * G) - 4.0 * np.sum(bm * bm))
    total = np.float32(np.sum(np.asarray(partials, dtype=np.float64)) / (B * N))
    return np.asarray(total, dtype=np.float32), res


def kernel(embeddings: np.ndarray, assignments: np.ndarray) -> np.ndarray:
    out, _ = _run(embeddings, assignments, trace=False)
    return out
